# revision 1
# baseline (speedup 1.0000x reference)
"""CARAFE upsampling kernel for 8 Trainium2 NeuronCores.

Problem (hardcoded): features (2,256,128,128) f32, masks (2,25,256,256) f32,
out (2,256,256,256) f32.  K=5, G=1, scale=2 (CARAFE content-aware upsample).

Strategy
--------
Sharding: 8 cores = batch(2) x H-half(2) x W-half(2).  Each core owns the
full C=256 and a 64x64 source patch (128x128 output patch) with a 2-pixel
feature halo (sliced with halo / zero-padded on host).

Compute: the 25-tap dynamic-filter sum becomes PSUM-accumulated TensorE
matmuls.  For source-row pair si, channel half ch, tap row dy, and output
column half h:

    out[c, (a, jj)] += featT[x', y=si+dy, c]^T  @  band[x', (a, jj)]

The band matrix (built host-side) holds mask values along x = jj//2 + dx
diagonals, zeros elsewhere.  Splitting the j-range in half (h) keeps the
contraction at K=36 (32 + 4 halo) instead of 68, nearly halving the band
bytes shipped from HBM.  The h=1 window (x in [32,68)) is x-REVERSED on the
host so both halves contract at partition base 0 (the contraction sum is
order-invariant); partition-offset matmuls crash the runtime here.  Each
matmul writes a contiguous 128-col PSUM slice (h-major); a single
start=True per PSUM bank clears has_written for the whole bank, so h=1's
first matmul (start=False) lands on cleared bits and overwrites.  M=128
channels, N=128 pixels/matmul; fp16 operands (rel err ~3.5e-4), fp32 PSUM.

Output leaves in native (c, i, j) layout via a PSUM->SBUF copy that
unscrambles (h, a, jj) -> (a, j), staged 8 si at a time into 1 MiB stores.
DMAs are batched per 8-si group to amortize HWDGE/sequencer fixed costs;
features become fully SBUF-resident by group 2.

TimelineSim cost model: ~82 us/core; PE busy ~70 us, DMA ~70 us.
"""

import numpy as np

import concourse.bacc as bacc
import concourse.bass as bass
import concourse.mybir as mybir
import concourse.tile as tile
from concourse.bass_utils import run_bass_kernel_spmd

FP16 = mybir.dt.float16
F32 = mybir.dt.float32

N_CORES = 8
C = 256
SI = 64
SX = 64
YR = SI + 4
XW = 36
OI = 2 * SI
OJ = 2 * SX
GROUP = 8
NG = SI // GROUP

_CACHED_NC = None
TRACE = False
_LAST_RESULTS = None


def _build_nc():
    nc = bacc.Bacc(None, target_bir_lowering=False, debug=False)

    # [hblock, x', y, c]: block 0 = x 0:36; block 1 = x 67..32 (reversed)
    featT_d = nc.dram_tensor("featT", [2, XW, YR, C], FP16, kind="ExternalInput")
    # [h, x', si, dy, (a jj)]; h=1 x'-axis reversed to match featT block 1
    bands_d = nc.dram_tensor("bands", [2, XW, SI, 5, 128], FP16, kind="ExternalInput")
    out_d = nc.dram_tensor("out", [C, OI, OJ], F32, kind="ExternalOutput")

    with tile.TileContext(nc) as tc:
        with (
            tc.tile_pool(name="feat", bufs=1) as fpool,
            tc.tile_pool(name="bands", bufs=3) as bpool,
            tc.tile_pool(name="psum", bufs=8, space=bass.MemorySpace.PSUM) as ppool,
            tc.tile_pool(name="stage", bufs=4) as spool,
        ):
            ftiles = [
                fpool.tile([XW, YR * C], FP16, tag=f"ft{h}", name=f"ft{h}")
                for h in range(2)
            ]
            fdone = [0, 0]

            def load_feat_rows(upto, h):
                nonlocal fdone
                if upto <= fdone[h]:
                    return
                nc.sync.dma_start(
                    ftiles[h][:, fdone[h] * C : upto * C],
                    featT_d[h, :, fdone[h] : upto].rearrange("x y c -> x (y c)"),
                )
                fdone[h] = upto

            for g in range(NG):
                btiles = [
                    bpool.tile([XW, GROUP * 640], FP16, tag=f"bt{h}", name=f"bt{h}")
                    for h in range(2)
                ]
                for h in range(2):
                    # group 0: halve the first transfers so matmuls start early
                    # (per-DMA fixed costs dominate here, so only one split)
                    splits = [(0, 4), (4, 8)] if g == 0 else [(0, GROUP)]
                    for s0, s1 in splits:
                        src = bands_d[h, :, g * GROUP + s0 : g * GROUP + s1]
                        nc.sync.dma_start(
                            btiles[h][:, s0 * 640 : s1 * 640],
                            src.rearrange("x s d j -> x (s d j)"),
                        )
                        if g == 0:
                            load_feat_rows(s1 + 4, h)
                    if g == 0:
                        load_feat_rows(GROUP * 2 + 4, h)
                    else:
                        load_feat_rows(YR, h)
                for ch in range(2):
                    stg = spool.tile([128, GROUP * 256], F32)
                    for sl in range(GROUP):
                        si = g * GROUP + sl
                        # psum columns are h-major (contiguous per matmul);
                        # the copy below unscrambles to (a, j) order
                        ps = ppool.tile([128, 256], F32)
                        for dy in range(5):
                            yoff = (si + dy) * C + ch * 128
                            for h in range(2):
                                boff = sl * 640 + dy * 128
                                # start once per bank (clears has_written for
                                # the WHOLE bank); h=1's first write lands on
                                # cleared bits and overwrites
                                nc.tensor.matmul(
                                    ps[:, 128 * h : 128 * h + 128],
                                    ftiles[h][:, yoff : yoff + 128],
                                    btiles[h][:, boff : boff + 128],
                                    start=(dy == 0 and h == 0),
                                    stop=(dy == 4 and h == 1),
                                    skip_group_check=True,
                                )
                        # stg col = a*128 + h*64 + jj, iterated in the psum's
                        # (h, a, jj) source order
                        dst = stg[:, sl * 256 : (sl + 1) * 256].rearrange(
                            "p (a h j) -> p h a j", a=2, h=2
                        )
                        if ch == 0:
                            nc.vector.tensor_copy(dst, ps[:])
                        else:
                            nc.scalar.copy(dst, ps[:])
                    odst = out_d[
                        ch * 128 : (ch + 1) * 128,
                        g * 2 * GROUP : (g + 1) * 2 * GROUP,
                        :,
                    ]
                    nc.scalar.dma_start(odst.rearrange("c a j -> c (a j)"), stg[:])

    nc.compile()
    return nc


def _get_nc():
    global _CACHED_NC
    if _CACHED_NC is None:
        _CACHED_NC = _build_nc()
    return _CACHED_NC


def _prep_core_inputs(features: np.ndarray, masks: np.ndarray):
    fp = np.pad(features, ((0, 0), (0, 0), (2, 2), (2, 2)))

    sjl = np.arange(32)
    in_maps = []
    for core in range(N_CORES):
        n, hb, wb = core // 4, (core // 2) % 2, core % 2

        fsl = fp[n, :, hb * SI : hb * SI + YR, wb * SX : wb * SX + SX + 4]
        featT = np.ascontiguousarray(fsl.transpose(2, 1, 0))  # (x, y, c)
        fA = featT[0:36]
        fB = featT[32:68][::-1]  # x reversed
        featT2 = np.stack([fA, fB]).astype(np.float16)

        msl = masks[n, :, hb * OI : (hb + 1) * OI, wb * OJ : (wb + 1) * OJ]
        m6 = msl.reshape(5, 5, SI, 2, SX, 2)
        bh = np.zeros((2, XW, SI, 5, 2, 32, 2), dtype=np.float32)
        for h in range(2):
            for dx in range(5):
                mh = m6[:, dx].transpose(3, 1, 0, 2, 4)[32 * h : 32 * h + 32]
                bh[h, sjl + dx, :, :, :, sjl, :] = mh
        bh[1] = bh[1][::-1]  # mirror x' to match featT block 1
        bands = bh.astype(np.float16).reshape(2, XW, SI, 5, 128)

        in_maps.append({"featT": featT2, "bands": bands})
    return in_maps


def kernel(features: np.ndarray, masks: np.ndarray) -> np.ndarray:
    global _LAST_RESULTS
    features = np.asarray(features, dtype=np.float32)
    masks = np.asarray(masks, dtype=np.float32)

    nc = _get_nc()
    in_maps = _prep_core_inputs(features, masks)
    res = run_bass_kernel_spmd(nc, in_maps, list(range(N_CORES)), trace=TRACE)
    _LAST_RESULTS = res

    out = np.empty((2, C, 256, 256), dtype=np.float32)
    for core in range(N_CORES):
        n, hb, wb = core // 4, (core // 2) % 2, core % 2
        out[n, :, hb * OI : (hb + 1) * OI, wb * OJ : (wb + 1) * OJ] = res.results[
            core
        ]["out"]
    return out



# revision 4
# speedup vs baseline: 1.2060x; 1.2060x over previous
"""CARAFE upsampling kernel for 8 Trainium2 NeuronCores.

Problem (hardcoded): features (2,256,128,128) f32, masks (2,25,256,256) f32,
out (2,256,256,256) f32.  K=5, G=1, scale=2 (CARAFE content-aware upsample).

Strategy
--------
Sharding: 8 cores = batch(2) x H-half(2) x W-half(2).  Each core owns the
full C=256 and a 64x64 source patch (128x128 output patch) with a 2-pixel
feature halo.

Compute: the 25-tap dynamic-filter sum becomes PSUM-accumulated TensorE
matmuls with the dy taps FOLDED IN PAIRS into the contraction dim
(K = 2dy x 36x' = 72), cutting streamed PSUM columns from 10x128 to 6x128
per (si, ch) tile (PE 68us -> 41us).  The feature operand is a [72, (y,c)]
SBUF tile whose two 36-partition blocks hold y-shifted copies of the
transposed features (block d = featT rows y+d, duplicated host-side so the
load is one plain base-0 DMA); one tile serves all si via a column offset.
dy groups are {0,1}, {2,3}, {4,pad} (pad rows multiply always-zero band
columns).

The banded mask operand is only partially expanded:
 - d=0 rows (partitions 0:36) are shipped COMPACT (60S-element runs) and
   scattered on-chip by one DMA per (h, si-group) whose dst access pattern
   has the diagonal stride RB+12S — partition+1, column+12S per x'.  This
   only lowers correctly at partition base 0 with offset 0 (nonzero bases
   crash the NEFF lowering), which both h halves satisfy: h=1 stores its
   columns in REVERSED-ssj order so its diagonal also starts at (0,0); the
   resulting j-reversal of the second output half is undone on the host.
 - d=1 rows (partitions 36:72) cannot use the diagonal (base-36 APs are
   rejected), so their read-region [48S,432S) is shipped dense (~3.5 MB)
   and loaded with a plain partition-offset DMA (allowed for standard APs).
The scatter support is si-independent, so each band buffer's d=0 rows are
memset once at startup and every group overwrites exactly the same cells.

Output is cast to fp16 in the PSUM->SBUF unscramble copy (rel-err budget
2e-2 dwarfs fp16 rounding; ~13us saved) and upcast to f32 on the host.

TimelineSim cost model: DMA ~49us (feat 5.1 + bands 4.1 + out 8.4 MB),
PE ~41us, DVE/Act copies ~25us each -> ~52us/core vs 82us baseline.
"""

import numpy as np

import concourse.bacc as bacc
import concourse.bass as bass
import concourse.mybir as mybir
import concourse.tile as tile
from concourse.bass_utils import run_bass_kernel_spmd

FP16 = mybir.dt.float16
F32 = mybir.dt.float32

N_CORES = 8
C = 256
SI = 64          # source rows per core
SX = 64          # source cols per core
YR = 68          # real featT y rows (64 + 4 halo)
YD = 70          # featT y rows incl. zero-pad rows for the d=1 block
YC = 69          # y columns per feature d-block
XW = 36          # x' per h half
OI = 2 * SI
OJ = 2 * SX
S = 8            # si per band group
NSIG = SI // S
RUN = 60 * S     # d0 scatter run elems: 5u * (2a * 2b * 3g * S)
RB = 480 * S     # band tile row elems: 40cb * 12S
CB0 = 4          # first read column block (ssj = sj + 4)
NCB = 32         # read column blocks

_CACHED_NC = None
TRACE = False
_LAST_RESULTS = None


def _build_nc():
    nc = bacc.Bacc(None, target_bir_lowering=False, debug=False)

    # host-duplicated features: [h, 36d+x', y, c] = featT[h, x', y+d, c]
    featT_d = nc.dram_tensor("featT", [2, 72, YC, C], FP16, kind="ExternalInput")
    # d=0 compact band runs, ordered (x', (u, a, b, g, s))
    packed_d = nc.dram_tensor("packed", [2, NSIG, XW, RUN], FP16,
                              kind="ExternalInput")
    # d=1 dense read-region: (x', cb in [4,36), (a, b, g, s))
    exp1_d = nc.dram_tensor("exp1", [2, NSIG, XW, NCB * 12 * S], FP16,
                            kind="ExternalInput")
    out_d = nc.dram_tensor("out", [C, OI, OJ], FP16, kind="ExternalOutput")

    with tile.TileContext(nc) as tc:
        with (
            tc.tile_pool(name="feat", bufs=1) as fpool,
            tc.tile_pool(name="bands", bufs=2) as bpool,
            tc.tile_pool(name="psum", bufs=8, space=bass.MemorySpace.PSUM) as ppool,
            tc.tile_pool(name="stage", bufs=4) as spool,
        ):
            ftiles = [
                fpool.tile([72, YC * C], FP16, tag=f"ft{h}", name=f"ft{h}")
                for h in range(2)
            ]
            fdone = [0, 0]

            def load_feat_rows(upto, h):
                nonlocal fdone
                if upto <= fdone[h]:
                    return
                nc.sync.dma_start(
                    ftiles[h][:, fdone[h] * C : upto * C],
                    featT_d[h, :, fdone[h] : upto].rearrange("x y c -> x (y c)"),
                )
                fdone[h] = upto

            for h in range(2):
                load_feat_rows(14, h)

            for sig in range(NSIG):
                btiles = [
                    bpool.tile([72, RB], FP16, tag=f"bt{h}", name=f"bt{h}")
                    for h in range(2)
                ]
                if sig < 2:
                    # zero d=0 rows of each physical buffer once; the scatter
                    # support is si-independent so zeros stay valid forever
                    nc.vector.memset(btiles[0][0:36, :], 0.0)
                    nc.gpsimd.memset(btiles[1][0:36, :], 0.0)
                for h in range(2):
                    dst = bass.AP(
                        btiles[h].tensor,
                        btiles[h].offset,
                        [[RB + 12 * S, XW], [1, RUN]],
                    )
                    nc.sync.dma_start(dst, packed_d[h, sig])
                    nc.sync.dma_start(
                        btiles[h][36:72, 12 * S * CB0 : 12 * S * (CB0 + NCB)],
                        exp1_d[h, sig],
                    )
                upto = min(YC, S * (sig + 1) + 6)
                for h in range(2):
                    load_feat_rows(upto, h)
                for ch in range(2):
                    stg = spool.tile([128, S * 256], FP16)
                    for sl in range(S):
                        si = sig * S + sl
                        ps = ppool.tile([128, 256], F32)
                        for g in range(3):
                            yoff = (si + 2 * g) * C + ch * 128
                            for h in range(2):
                                rhs = bass.AP(
                                    btiles[h].tensor,
                                    btiles[h].offset + 48 * S + g * S + sl,
                                    [[RB, 72], [6 * S, 2], [12 * S, 32], [3 * S, 2]],
                                )
                                nc.tensor.matmul(
                                    ps[:, 128 * h : 128 * h + 128],
                                    ftiles[h][:, yoff : yoff + 128],
                                    rhs,
                                    start=(g == 0 and h == 0),
                                    stop=(g == 2 and h == 1),
                                    skip_group_check=True,
                                )
                        # psum cols (h, a, sj, b) -> stage cols a*128+h*64+sj*2+b
                        # (h=1 j-order is host-fixed)
                        dst = stg[:, sl * 256 : (sl + 1) * 256].rearrange(
                            "p (a h s b) -> p h a s b", a=2, h=2, s=32
                        )
                        if ch == 0:
                            nc.vector.tensor_copy(dst, ps[:])
                        else:
                            nc.scalar.copy(dst, ps[:])
                    odst = out_d[
                        ch * 128 : (ch + 1) * 128,
                        sig * 2 * S : (sig + 1) * 2 * S,
                        :,
                    ]
                    nc.scalar.dma_start(odst.rearrange("c a j -> c (a j)"), stg[:])

    nc.compile()
    return nc


def _get_nc():
    global _CACHED_NC
    if _CACHED_NC is None:
        _CACHED_NC = _build_nc()
    return _CACHED_NC


def _band_value_indices():
    """Index grids (core-independent) for band values.

    Returns (k25, ii_a, jj, valid) grids as functions of
    (h, d, x', u', a, b, g, s) where u' = cb - x':
      dy = 2g + d, h=0: sj = x'+u'-4, dx = 4-u', j = 2sj+b
                   h=1: sjr = x'+u'-4, dx = u', j = 64 + 2(31-sjr) + b
    """
    h_ = np.arange(2)[:, None, None, None, None, None, None, None]
    d_ = np.arange(2)[None, :, None, None, None, None, None, None]
    x_ = np.arange(XW)[None, None, :, None, None, None, None, None]
    u_ = np.arange(5)[None, None, None, :, None, None, None, None]
    a_ = np.arange(2)[None, None, None, None, :, None, None, None]
    b_ = np.arange(2)[None, None, None, None, None, :, None, None]
    g_ = np.arange(3)[None, None, None, None, None, None, :, None]
    s_ = np.arange(S)[None, None, None, None, None, None, None, :]
    dy = 2 * g_ + d_
    sj0 = x_ + u_ - 4                     # h=0 sj / h=1 sjr
    sj = np.where(h_ == 0, sj0, 31 - sj0)  # true source col within half
    dx = np.where(h_ == 0, 4 - u_, u_)
    valid = (dy <= 4) & (sj0 >= 0) & (sj0 < 32)
    k25 = np.where(valid, 5 * np.minimum(dy, 4) + dx, 0)
    jj = h_ * 64 + 2 * np.clip(sj, 0, 31) + b_
    bc = np.broadcast_shapes(k25.shape, a_.shape, jj.shape, s_.shape)
    return (np.broadcast_to(k25, bc), np.broadcast_to(a_, bc),
            np.broadcast_to(jj, bc), np.broadcast_to(s_, bc),
            np.broadcast_to(valid, bc))


_K25, _AB, _JB, _SB, _VB = _band_value_indices()


def _prep_core_inputs(features: np.ndarray, masks: np.ndarray):
    fp = np.pad(features, ((0, 0), (0, 0), (2, 2), (2, 2)))

    in_maps = []
    for core in range(N_CORES):
        n, hb, wb = core // 4, (core // 2) % 2, core % 2

        fsl = fp[n, :, hb * SI : hb * SI + YR, wb * SX : wb * SX + SX + 4]
        featT = np.ascontiguousarray(fsl.transpose(2, 1, 0))  # (x, y, c)
        fA = featT[0:36]
        fB = featT[32:68][::-1]  # x reversed
        f2 = np.stack([fA, fB]).astype(np.float16)            # [2, 36, 68, C]
        f2 = np.pad(f2, ((0, 0), (0, 0), (0, YD - YR), (0, 0)))  # [2,36,70,C]
        # duplicate into d-blocks: [h, 36d+x', y, c] = f2[h, x', y+d, c]
        featT2 = np.concatenate([f2[:, :, 0:YC], f2[:, :, 1 : 1 + YC]], axis=1)

        msl = masks[n, :, hb * OI : (hb + 1) * OI, wb * OJ : (wb + 1) * OJ]
        # values[h, sig, d, x', u', a, b, g, s]
        vals = np.zeros((2, NSIG, 2, XW, 5, 2, 2, 3, S), dtype=np.float16)
        for sig in range(NSIG):
            ii = 2 * (sig * S + _SB) + _AB
            v = msl[_K25, ii, _JB].astype(np.float16)
            vals[:, sig][_VB] = v[_VB]
        packed = np.ascontiguousarray(vals[:, :, 0]).reshape(2, NSIG, XW, RUN)
        # d=1 dense read-region: [h, sig, x', cb-4, a, b, g, s]
        exp1 = np.zeros((2, NSIG, XW, NCB, 2, 2, 3, S), dtype=np.float16)
        x_ = np.arange(XW)[:, None]
        u_ = np.arange(5)[None, :]
        cb = x_ + u_                      # column block of run element u'
        sel = (cb >= CB0) & (cb < CB0 + NCB)
        xi, ui = np.nonzero(sel)
        exp1[:, :, xi, cb[xi, ui] - CB0] = vals[:, :, 1, xi, ui]
        exp1 = exp1.reshape(2, NSIG, XW, NCB * 12 * S)

        in_maps.append({"featT": featT2, "packed": packed, "exp1": exp1})
    return in_maps


def kernel(features: np.ndarray, masks: np.ndarray) -> np.ndarray:
    global _LAST_RESULTS
    features = np.asarray(features, dtype=np.float32)
    masks = np.asarray(masks, dtype=np.float32)

    nc = _get_nc()
    in_maps = _prep_core_inputs(features, masks)
    res = run_bass_kernel_spmd(nc, in_maps, list(range(N_CORES)), trace=TRACE)
    _LAST_RESULTS = res

    out = np.empty((2, C, 256, 256), dtype=np.float32)
    for core in range(N_CORES):
        n, hb, wb = core // 4, (core // 2) % 2, core % 2
        o = res.results[core]["out"].astype(np.float32)
        # undo the reversed-sjr order of the second j half
        right = o[:, :, 64:].reshape(C, OI, 32, 2)
        o[:, :, 64:] = right[:, :, ::-1, :].reshape(C, OI, 64)
        out[n, :, hb * OI : (hb + 1) * OI, wb * OJ : (wb + 1) * OJ] = o
    return out


# revision 21
# speedup vs baseline: 1.4527x; 1.2046x over previous
"""CARAFE upsampling kernel for 8 Trainium2 NeuronCores.

Problem (hardcoded): features (2,256,128,128) f32, masks (2,25,256,256) f32,
out (2,256,256,256) f32.  K=5, G=1, scale=2 (CARAFE content-aware upsample).

Strategy
--------
Sharding: 8 cores = batch(2) x H-half(2) x W-half(2).  Each core owns the
full C=256 and a 64x64 source patch (128x128 output patch) with a 2-pixel
feature halo.

Compute: the 25-tap dynamic-filter sum becomes PSUM-accumulated TensorE
matmuls with the dy taps FOLDED IN PAIRS into the contraction dim
(K = 2dy x 36x' = 72), cutting streamed PSUM columns from 10x128 to 6x128
per (si, ch) tile (PE 68us -> 41us).  The feature operand is a [72, (y,c)]
SBUF tile whose two 36-partition blocks hold y-shifted copies of the
transposed features (block d = featT rows y+d, duplicated host-side so the
load is one plain base-0 DMA); one tile serves all si via a column offset.
dy groups are {0,1}, {2,3}, {4,pad} (pad rows multiply always-zero band
columns).

The banded mask operand is only partially expanded:
 - d=0 rows (partitions 0:36) are shipped COMPACT (60S-element runs) and
   scattered on-chip by one DMA per (h, si-group) whose dst access pattern
   has the diagonal stride RB+12S — partition+1, column+12S per x'.  This
   only lowers correctly at partition base 0 with offset 0 (nonzero bases
   crash the NEFF lowering), which both h halves satisfy: h=1 stores its
   columns in REVERSED-ssj order so its diagonal also starts at (0,0); the
   resulting j-reversal of the second output half is undone on the host.
 - d=1 rows (partitions 36:72) cannot use the diagonal (base-36 APs are
   rejected), so their read-region [48S,432S) is shipped dense (~3.5 MB)
   and loaded with a plain partition-offset DMA (allowed for standard APs).
The scatter support is si-independent, so each band buffer's d=0 rows are
memset once at startup and every group overwrites exactly the same cells.

Output is cast to fp16 in the PSUM->SBUF unscramble copy (rel-err budget
2e-2 dwarfs fp16 rounding; ~13us saved) and upcast to f32 on the host.

TimelineSim cost model: DMA ~49us (feat 5.1 + bands 4.1 + out 8.4 MB),
PE ~41us, DVE/Act copies ~25us each -> ~52us/core vs 82us baseline.
"""

import numpy as np

import concourse.bacc as bacc
import concourse.bass as bass
import concourse.mybir as mybir
import concourse.tile as tile
from concourse.bass_utils import run_bass_kernel_spmd

FP16 = mybir.dt.float16
F32 = mybir.dt.float32

N_CORES = 8
C = 256
SI = 64          # source rows per core
SX = 64          # source cols per core
YR = 68          # real featT y rows (64 + 4 halo)
YD = 70          # featT y rows incl. zero-pad rows for the d=1 block
YC = 69          # y columns per feature d-block
XW = 36          # x' per h half
OI = 2 * SI
OJ = 2 * SX
S = 8            # si per band group
NSIG = SI // S
RUN = 60 * S     # d0 scatter run elems: 5u * (2a * 2b * 3g * S)
RB = 480 * S     # band tile row elems: 40cb * 12S
CB0 = 4          # first read column block (ssj = sj + 4)
NCB = 32         # read column blocks

_CACHED_NC = None
TRACE = False
_LAST_RESULTS = None


def _build_nc():
    nc = bacc.Bacc(None, target_bir_lowering=False, debug=False)

    # host-duplicated features: [h, 36d+x', y, c] = featT[h, x', y+d, c]
    featT_d = nc.dram_tensor("featT", [2, 72, YC, C], FP16, kind="ExternalInput")
    # d=0 compact band runs, ordered (x', g, (u, a, b, s))
    packed_d = nc.dram_tensor("packed", [2, NSIG, XW, RUN], FP16,
                              kind="ExternalInput")
    # d=1 dense read-region, g-major: (x', g in {0,1}, cb in [4,36), (a, b, s))
    exp1_d = nc.dram_tensor("exp1", [2, NSIG, XW, 2 * NCB * 4 * S], FP16,
                            kind="ExternalInput")
    # sig-0 band images with zeros baked in (skips the startup memset gate)
    band0_d = nc.dram_tensor("band0", [2, 72, RB], FP16, kind="ExternalInput")
    out_d = nc.dram_tensor("out", [C, OI, OJ], FP16, kind="ExternalOutput")

    with tile.TileContext(nc) as tc:
        with (
            tc.tile_pool(name="feat", bufs=1) as fpool,
            tc.tile_pool(name="bands", bufs=4) as bpool,
            tc.tile_pool(name="psum", bufs=8, space=bass.MemorySpace.PSUM) as ppool,
            tc.tile_pool(name="stage", bufs=6) as spool,
        ):
            ftiles = [
                fpool.tile([72, YC * C], FP16, tag=f"ft{h}", name=f"ft{h}")
                for h in range(2)
            ]
            fdone = [0, 0]

            def load_feat_rows(upto, h):
                nonlocal fdone
                if upto <= fdone[h]:
                    return
                nc.scalar.dma_start(
                    ftiles[h][:, fdone[h] * C : upto * C],
                    featT_d[h, :, fdone[h] : upto].rearrange("x y c -> x (y c)"),
                )
                fdone[h] = upto

            # --- startup: preload sig-0 bands (zeros baked into the DRAM
            # image) interleaved with the first feature chunks; memset the
            # other three band buffers off the critical path ---
            bbufs = [
                [bpool.tile([72, RB], FP16, tag=f"bt{h}", name=f"bt{h}{k}")
                 for h in range(2)]
                for k in range(4)
            ]
            load_feat_rows(12, 0)
            nc.sync.dma_start(bbufs[0][0][:, :], band0_d[0])
            load_feat_rows(12, 1)
            nc.sync.dma_start(bbufs[0][1][:, :], band0_d[1])
            # B zeroed fast (DVE + Pool); C on Pool (free until sig-0 preps)
            nc.vector.memset(bbufs[1][0][:, :], 0.0)
            nc.gpsimd.memset(bbufs[1][1][:, :], 0.0)
            nc.gpsimd.memset(bbufs[2][0][:, :], 0.0)
            nc.gpsimd.memset(bbufs[2][1][:, :], 0.0)

            for sig in range(NSIG):
                btiles = bbufs[sig % 4]
                if sig > 0:
                    for h in range(2):
                        dst = bass.AP(
                            btiles[h].tensor,
                            btiles[h].offset,
                            [[RB + 4 * S, XW], [160 * S, 3], [1, 20 * S]],
                        )
                        nc.sync.dma_start(
                            dst,
                            packed_d[h, sig].rearrange("x (g r) -> x g r", g=3))
                        edst = bass.AP(
                            btiles[h].tensor,
                            btiles[h].offset + 36 * RB + 16 * S,
                            [[RB, XW], [160 * S, 2], [1, 128 * S]],
                        )
                        nc.sync.dma_start(
                            edst, exp1_d[h, sig].rearrange("x (g r) -> x g r", g=2))
                if sig == 1:
                    # zero the 4th buffer (first used at sig 3)
                    nc.vector.memset(bbufs[3][0][:, :], 0.0)
                    nc.gpsimd.memset(bbufs[3][1][:, :], 0.0)
                if sig in (1, 3, 5):
                    upto = min(YC, S * (sig + 3) + 6)
                    for h in range(2):
                        load_feat_rows(upto, h)
                last = sig == NSIG - 1
                if last:
                    # separate per-half tiles so each out store depends only
                    # on its own four copies (shorter tail)
                    stgh = [[spool.tile([128, S * 128], FP16, bufs=1,
                                        name=f"stgh{ch}{half}")
                             for half in range(2)] for ch in range(2)]
                else:
                    stgs = [spool.tile([128, S * 256], FP16, name=f"stg{ch}")
                            for ch in range(2)]

                def issue_mm(ps, si, sl, ch, g):
                    yoff = (si + 2 * g) * C + ch * 128
                    for h in range(2):
                        rhs = bass.AP(
                            btiles[h].tensor,
                            btiles[h].offset + g * 160 * S + 16 * S + sl,
                            [[RB, 72], [2 * S, 2], [4 * S, 32], [S, 2]],
                        )
                        nc.tensor.matmul(
                            ps[:, 128 * h : 128 * h + 128],
                            ftiles[h][:, yoff : yoff + 128],
                            rhs,
                            start=(g == 0 and h == 0),
                            stop=(g == 2 and h == 1),
                            skip_group_check=True,
                        )

                def do_copy(ps, sl, ch):
                    # psum cols (h, a, sj, b) -> stage cols a*128+h*64+sj*2+b
                    # (h=1 j-order is host-fixed)
                    if last:
                        st = stgh[ch][sl // (S // 2)]
                        c0 = (sl % (S // 2)) * 256
                    else:
                        st = stgs[ch]
                        c0 = sl * 256
                    dst = st[:, c0 : c0 + 256].rearrange(
                        "p (a h s b) -> p h a s b", a=2, h=2, s=32
                    )
                    if ch == 0:
                        nc.vector.tensor_copy(dst, ps[:])
                    else:
                        nc.scalar.copy(dst, ps[:])

                if True:
                    for sl in range(S):
                        si = sig * S + sl
                        for ch in range(2):
                            ps = ppool.tile([128, 256], F32, name="ps",
                                            tag="ps")
                            for g in range(3):
                                issue_mm(ps, si, sl, ch, g)
                            do_copy(ps, sl, ch)

                if last:
                    for half in range(2):
                        r0 = sig * 2 * S + half * S
                        for ch in range(2):
                            odst = out_d[ch * 128 : (ch + 1) * 128,
                                         r0 : r0 + S, :]
                            nc.sync.dma_start(
                                odst.rearrange("c a j -> c (a j)"),
                                stgh[ch][half][:])
                else:
                    for ch in range(2):
                        odst = out_d[
                            ch * 128 : (ch + 1) * 128,
                            sig * 2 * S : (sig + 1) * 2 * S,
                            :,
                        ]
                        nc.gpsimd.dma_start(
                            odst.rearrange("c a j -> c (a j)"), stgs[ch][:])

    nc.compile()
    return nc


def _get_nc():
    global _CACHED_NC
    if _CACHED_NC is None:
        _CACHED_NC = _build_nc()
    return _CACHED_NC


def _band_value_indices():
    """Index grids (core-independent) for band values.

    Returns (k25, ii_a, jj, valid) grids as functions of
    (h, d, x', u', a, b, g, s) where u' = cb - x':
      dy = 2g + d, h=0: sj = x'+u'-4, dx = 4-u', j = 2sj+b
                   h=1: sjr = x'+u'-4, dx = u', j = 64 + 2(31-sjr) + b
    """
    h_ = np.arange(2)[:, None, None, None, None, None, None, None]
    d_ = np.arange(2)[None, :, None, None, None, None, None, None]
    x_ = np.arange(XW)[None, None, :, None, None, None, None, None]
    u_ = np.arange(5)[None, None, None, :, None, None, None, None]
    a_ = np.arange(2)[None, None, None, None, :, None, None, None]
    b_ = np.arange(2)[None, None, None, None, None, :, None, None]
    g_ = np.arange(3)[None, None, None, None, None, None, :, None]
    s_ = np.arange(S)[None, None, None, None, None, None, None, :]
    dy = 2 * g_ + d_
    sj0 = x_ + u_ - 4                     # h=0 sj / h=1 sjr
    sj = np.where(h_ == 0, sj0, 31 - sj0)  # true source col within half
    dx = np.where(h_ == 0, 4 - u_, u_)
    valid = (dy <= 4) & (sj0 >= 0) & (sj0 < 32)
    k25 = np.where(valid, 5 * np.minimum(dy, 4) + dx, 0)
    jj = h_ * 64 + 2 * np.clip(sj, 0, 31) + b_
    bc = np.broadcast_shapes(k25.shape, a_.shape, jj.shape, s_.shape)
    return (np.broadcast_to(k25, bc), np.broadcast_to(a_, bc),
            np.broadcast_to(jj, bc), np.broadcast_to(s_, bc),
            np.broadcast_to(valid, bc))


_K25, _AB, _JB, _SB, _VB = _band_value_indices()


def _prep_core_inputs(features: np.ndarray, masks: np.ndarray):
    fp = np.pad(features, ((0, 0), (0, 0), (2, 2), (2, 2)))

    in_maps = []
    for core in range(N_CORES):
        n, hb, wb = core // 4, (core // 2) % 2, core % 2

        fsl = fp[n, :, hb * SI : hb * SI + YR, wb * SX : wb * SX + SX + 4]
        featT = np.ascontiguousarray(fsl.transpose(2, 1, 0))  # (x, y, c)
        fA = featT[0:36]
        fB = featT[32:68][::-1]  # x reversed
        f2 = np.stack([fA, fB]).astype(np.float16)            # [2, 36, 68, C]
        f2 = np.pad(f2, ((0, 0), (0, 0), (0, YD - YR), (0, 0)))  # [2,36,70,C]
        # duplicate into d-blocks: [h, 36d+x', y, c] = f2[h, x', y+d, c]
        featT2 = np.concatenate([f2[:, :, 0:YC], f2[:, :, 1 : 1 + YC]], axis=1)

        msl = masks[n, :, hb * OI : (hb + 1) * OI, wb * OJ : (wb + 1) * OJ]
        # values[h, sig, d, x', u', a, b, g, s]
        vals = np.zeros((2, NSIG, 2, XW, 5, 2, 2, 3, S), dtype=np.float16)
        for sig in range(NSIG):
            ii = 2 * (sig * S + _SB) + _AB
            v = msl[_K25, ii, _JB].astype(np.float16)
            vals[:, sig][_VB] = v[_VB]
        # g-major run layout: [h, sig, x', g, u, a, b, s]
        packed4 = vals[:, :, 0].transpose(0, 1, 2, 6, 3, 4, 5, 7)
        packed = np.ascontiguousarray(packed4).reshape(2, NSIG, XW, RUN)
        # d=1 dense read-region, g-major: [h, sig, x', g in {0,1}, cb-4, a, b, s]
        exp1 = np.zeros((2, NSIG, XW, 2, NCB, 2, 2, S), dtype=np.float16)
        x_ = np.arange(XW)[:, None]
        u_ = np.arange(5)[None, :]
        cb = x_ + u_                      # column block of run element u'
        sel = (cb >= CB0) & (cb < CB0 + NCB)
        xi, ui = np.nonzero(sel)
        v1 = vals[:, :, 1].transpose(0, 1, 2, 6, 3, 4, 5, 7)  # [h,sig,x',g,u,a,b,s]
        exp1[:, :, xi, :, cb[xi, ui] - CB0] = v1[:, :, xi, 0:2, ui]
        exp1f = exp1.reshape(2, NSIG, XW, 2 * NCB * 4 * S)

        # sig-0 full band images (zeros baked in)
        band0 = np.zeros((2, 72, RB), dtype=np.float16)
        for g in range(3):
            for x in range(XW):
                c0 = g * 160 * S + x * 4 * S
                band0[:, x, c0 : c0 + 20 * S] = packed4[:, 0, x, g].reshape(2, -1)
        for g in range(2):
            c0 = g * 160 * S + 16 * S
            band0[:, 36:72, c0 : c0 + 128 * S] = exp1[:, 0, :, g].reshape(
                2, XW, 128 * S)

        in_maps.append({"featT": featT2, "packed": packed, "exp1": exp1f,
                        "band0": band0})
    return in_maps


def kernel(features: np.ndarray, masks: np.ndarray) -> np.ndarray:
    global _LAST_RESULTS
    features = np.asarray(features, dtype=np.float32)
    masks = np.asarray(masks, dtype=np.float32)

    nc = _get_nc()
    in_maps = _prep_core_inputs(features, masks)
    res = run_bass_kernel_spmd(nc, in_maps, list(range(N_CORES)), trace=TRACE)
    _LAST_RESULTS = res

    out = np.empty((2, C, 256, 256), dtype=np.float32)
    for core in range(N_CORES):
        n, hb, wb = core // 4, (core // 2) % 2, core % 2
        o = res.results[core]["out"].astype(np.float32)
        # undo the reversed-sjr order of the second j half
        right = o[:, :, 64:].reshape(C, OI, 32, 2)
        o[:, :, 64:] = right[:, :, ::-1, :].reshape(C, OI, 64)
        out[n, :, hb * OI : (hb + 1) * OI, wb * OJ : (wb + 1) * OJ] = o
    return out
